# revision 7
# baseline (speedup 1.0000x reference)
"""Banded multi-head attention kernel for Trainium2 (8 NeuronCores).

Problem: q = query @ Wq.T + bq, k = key @ Wk.T + bk  (per head, dk=64),
scores = q.k / sqrt(dk) masked to |i-j| <= 16, softmax over keys, then
gather the 33-column select window per row -> out [B, NH, T, 33].

Strategy (v3 = v2 + DMA/scheduling fixes):
  - Shard (batch b, half of T) across the 8 cores; each core computes all
    8 heads for its 1024 query rows.
  - All matmuls in float16 (1 cycle/col on the PE vs 4 for fp32
    LOW_HIGH multipass; fp32 PSUM accumulation).  End-to-end rel err
    ~1.5e-3 (verified on HW) vs the 2e-2 gate.
  - Scores in a 64-wide band: each 128-row block is split into four
    32-row strips whose key windows are 64 wide (vs 160 for the whole
    block).  8 heads x 4 strips = 32 matmuls [K=64,M=32,N=64] pack into
    one 512-col PSUM bank via 4-way column tiling and issue with ~4ns
    stagger (measured).  All score matmuls use PE array rows 0-63 only:
    mixing row groups across column-sharing matmuls faults the device
    (measured), so the odd heads' projections (PSUM partitions 64-127)
    are first moved to partitions 0-63 with SBUF->SBUF DMAs.
  - The band mask (-60000 outside the window) initializes the PSUM bank
    via one full matmul ident.T @ mask (start=True); the 32 strip
    matmuls accumulate on top (per-element has_written semantics).
    ScalarE does exp(x/8) PSUM->SBUF in ONE 512-wide op per block; DVE
    does one 512-wide row-sum reduce.  No per-head elementwise ops.
  - v3 scheduling: inputs packed into 3 big blobs + 2 small DMAs issued
    across BOTH hardware DGE queues (Sync + ScalarE) in first-use order
    (v2 lost 12us to serial DMA issue); 8 dummy warm-up matmuls on a
    memset tile run during the DMA lead-in so HAM un-throttles the PE
    clock before the real work; output-band DMAs alternate queues.
  - Normalization (divide by row sums) happens on the host, which also
    corrects the denominators of the <=16 global edge rows exactly
    (out-of-range keys are zero-padded -> each contributes exactly 1.0).
  - Host: final diagonal gather band -> [T, 33] + divide.
"""

import sys

sys.path.insert(0, "/opt/trn_rl_repo")

import numpy as np

B, T, HID = 4, 2048, 512
NH, DK, W = 8, 64, 16
WIN = 2 * W + 1  # 33
TEMP = 8.0
NCORES = 8
THALF = T // 2  # rows per core
NBLK = THALF // 128  # 8 row blocks per core
SB = 64  # strip band width: 32-row strip -> 32 + 2*16 keys
KW = THALF + 2 * W  # 1056 k^T columns needed per core
NEG = -60000.0  # band mask value (f16-representable; exp -> 0)
NWARM = 8  # HAM warm-up matmuls

_CACHE = {}


def _build_nc():
    import concourse.bass as bass  # noqa: F401
    import concourse.tile as tile
    from concourse import bacc, mybir

    f32 = mybir.dt.float32
    f16 = mybir.dt.float16
    AF = mybir.ActivationFunctionType
    AX = mybir.AxisListType
    ALU = mybir.AluOpType

    nc = bacc.Bacc("TRN2", target_bir_lowering=False, debug=False)

    # packed input blobs (fewer DMA issues; first-use order)
    # in1 = [wq | q cols 0:512], in2 = [wk | k cols 0:512],
    # in3 = [q cols 512:1024 | k cols 512:1056], sm = [mask | ident]
    in1 = nc.dram_tensor("in1", [128, 4, 1024], f16, kind="ExternalInput").ap()
    in2 = nc.dram_tensor("in2", [128, 4, 1024], f16, kind="ExternalInput").ap()
    in3 = nc.dram_tensor("in3", [128, 4, 1056], f16, kind="ExternalInput").ap()
    smi = nc.dram_tensor("smi", [128, 640], f16, kind="ExternalInput").ap()
    bia = nc.dram_tensor("bia", [128, 8], f32, kind="ExternalInput").ap()
    # exp band: [p, r, h, n]; row sums [p, r, h]
    outp = nc.dram_tensor(
        "outp", [128, NBLK, NH, SB], f32, kind="ExternalOutput"
    ).ap()
    sums = nc.dram_tensor("sums", [128, NBLK, NH], f32, kind="ExternalOutput").ap()

    with tile.TileContext(nc) as tc:
        from contextlib import ExitStack

        with ExitStack() as ctx:
            const = ctx.enter_context(tc.tile_pool(name="const", bufs=1))
            psum_p = ctx.enter_context(
                tc.tile_pool(name="psum_p", bufs=3, space="PSUM")
            )
            psum_s = ctx.enter_context(
                tc.tile_pool(name="psum_s", bufs=2, space="PSUM")
            )

            in1_sb = const.tile([128, 4, 1024], f16, tag="in1", name="in1")
            in2_sb = const.tile([128, 4, 1024], f16, tag="in2", name="in2")
            in3_sb = const.tile([128, 4, 1056], f16, tag="in3", name="in3")
            sm_sb = const.tile([128, 640], f16, tag="smi", name="smi")
            bia_sb = const.tile([128, 8], f32, tag="bia", name="bias")
            warm = const.tile([128, 512], f16, tag="wrm", name="wrm")
            # projections: [p = out-channel within oc chunk, oc, t]
            qp = const.tile([128, 4, THALF], f16, tag="qp", name="qp")
            kp = const.tile([128, 4, KW], f16, tag="kp", name="kp")
            # odd heads' dk rows moved to partitions 0-63
            qpo = const.tile([64, 4, THALF], f16, tag="qpo", name="qpo")
            kpo = const.tile([64, 4, KW], f16, tag="kpo", name="kpo")
            # persistent exp-band region [p, r, h, n] + row sums
            ob = const.tile([128, NBLK, NH, SB], f32, tag="ob", name="ob")
            rs_sb = const.tile([128, NBLK, NH], f32, tag="rs", name="rs")

            msk_ap = sm_sb[:, 0:512]
            idn_ap = sm_sb[:, 512:640]

            # HAM warm-up: dummy matmuls on a memset tile keep the PE busy
            # during the DMA lead-in so the clock gate opens before the
            # real matmuls start (cold PE runs at half clock).
            nc.gpsimd.memset(warm[:, :], 0.0)
            wps = psum_p.tile([128, 512], f32, tag="wps", name="wps")
            for _ in range(NWARM):
                nc.tensor.matmul(
                    wps[:, :], warm[:, 0:128], warm[:, :], start=True, stop=True
                )

            # input DMAs split across both hardware DGE queues,
            # ordered by first use
            nc.sync.dma_start(out=in1_sb[:, :, :], in_=in1[:, :, :])
            nc.scalar.dma_start(out=bia_sb[:, :], in_=bia[:, :])
            nc.scalar.dma_start(out=sm_sb[:, :], in_=smi[:, :])
            nc.scalar.dma_start(out=in2_sb[:, :, :], in_=in2[:, :, :])
            nc.sync.dma_start(out=in3_sb[:, :, :], in_=in3[:, :, :])

            ncopy = [0]

            def psum_to_sbuf(dst, ps_ap, bia_ap):
                # alternate psum->sbuf(+bias) copies between ScalarE and DVE
                if ncopy[0] % 2 == 0:
                    nc.scalar.activation(
                        dst, ps_ap, AF.Identity, bias=bia_ap, scale=1.0
                    )
                else:
                    nc.vector.tensor_scalar_add(dst, ps_ap, bia_ap)
                ncopy[0] += 1

            def emit_qproj(tb):
                tsl = slice(512 * tb, 512 * (tb + 1))
                for oc in range(4):
                    osl = slice(128 * oc, 128 * (oc + 1))
                    ps = psum_p.tile([128, 512], f32, tag="psp", name="psp")
                    for ic in range(4):
                        src = (
                            in1_sb[:, ic, 512:1024]
                            if tb == 0
                            else in3_sb[:, ic, 0:512]
                        )
                        nc.tensor.matmul(
                            ps[:, :],
                            (in1_sb[:, ic, osl]),
                            src,
                            start=(ic == 0),
                            stop=(ic == 3),
                        )
                    psum_to_sbuf(qp[:, oc, tsl], ps[:, :], bia_sb[:, oc : oc + 1])
                # odd heads' 64 dk rows -> partitions 0-63 (all 4 oc at once)
                nc.scalar.dma_start(out=qpo[:, :, tsl], in_=qp[64:128, :, tsl])

            def emit_kproj(c0, cn, src_of_ic):
                csl = slice(c0, c0 + cn)
                for oc in range(4):
                    osl = slice(128 * oc, 128 * (oc + 1))
                    ps = psum_p.tile([128, 512], f32, tag="psp", name="psp")
                    for ic in range(4):
                        nc.tensor.matmul(
                            ps[:, :cn],
                            in2_sb[:, ic, osl],
                            src_of_ic(ic),
                            start=(ic == 0),
                            stop=(ic == 3),
                        )
                    psum_to_sbuf(
                        kp[:, oc, csl], ps[:, :cn], bia_sb[:, 4 + oc : 5 + oc]
                    )
                nc.scalar.dma_start(out=kpo[:, :, csl], in_=kp[64:128, :, csl])

            def emit_scores(r):
                ps = psum_s.tile([128, NH * SB], f32, tag="pss", name="pss")
                # initialize the whole bank with the band mask via the PE
                # (ident.T @ mask, start=True); the 32 strip matmuls then
                # accumulate on top (per-element has_written adds onto the
                # mask).  skip_group_check: the sim's zero-region tracker
                # can't express partition-sliced accumulation, but
                # per-element HW semantics are exact (verified on HW).
                nc.tensor.matmul(
                    ps[:, :], idn_ap, msk_ap, start=True, stop=False,
                    skip_group_check=True,
                )
                nmm = 0
                for half in range(2):  # even heads first: qpo/kpo arrive later
                    for oc in range(4):
                        h = 2 * oc + half
                        for s in range(4):
                            c = 128 * r + 32 * s
                            if half == 0:
                                lhsT = qp[0:64, oc, c : c + 32]
                                rhs = kp[0:64, oc, c : c + SB]
                            else:
                                lhsT = qpo[:, oc, c : c + 32]
                                rhs = kpo[:, oc, c : c + SB]
                            nmm += 1
                            nc.tensor.matmul(
                                ps[32 * s : 32 * s + 32, SB * h : SB * (h + 1)],
                                lhsT,
                                rhs,
                                start=False,
                                stop=(nmm == 32),
                                tile_position=(0, 32 * s),
                                skip_group_check=True,
                            )
                nc.scalar.activation(
                    ob[:, r, :, :], ps[:, :], AF.Exp, scale=1.0 / TEMP
                )
                nc.vector.tensor_reduce(
                    rs_sb[:, r, :], ob[:, r, :, :], axis=AX.X, op=ALU.add
                )
                if r % 2 == 0:
                    nc.sync.dma_start(out=outp[:, r, :, :], in_=ob[:, r, :, :])
                else:
                    nc.scalar.dma_start(out=outp[:, r, :, :], in_=ob[:, r, :, :])

            # interleave projections and score blocks so ScalarE/DVE
            # post-processing overlaps PE matmuls throughout
            emit_qproj(0)
            emit_kproj(0, 512, lambda ic: in2_sb[:, ic, 512:1024])
            for r in range(0, 3):
                emit_scores(r)
            emit_qproj(1)
            emit_kproj(512, 512, lambda ic: in3_sb[:, ic, 512:1024])
            for r in range(3, 7):
                emit_scores(r)
            emit_kproj(1024, KW - 1024, lambda ic: in3_sb[:, ic, 1024:1056])
            emit_scores(7)
            nc.sync.dma_start(out=sums[:, :, :], in_=rs_sb[:, :, :])

    nc.compile()
    return nc


def _get_nc():
    if "nc" not in _CACHE:
        _CACHE["nc"] = _build_nc()
    return _CACHE["nc"]


def host_prep(query, key, Wq, bq, Wk, bk):
    """Build the 8 per-core input maps."""
    query = np.asarray(query, dtype=np.float32)
    key = np.asarray(key, dtype=np.float32)
    Wq = np.asarray(Wq, dtype=np.float32)
    Wk = np.asarray(Wk, dtype=np.float32)
    bq = np.asarray(bq, dtype=np.float32)
    bk = np.asarray(bk, dtype=np.float32)

    wqT = np.ascontiguousarray(Wq.T).astype(np.float16)  # [HID(in), HID(out)]
    wkT = np.ascontiguousarray(Wk.T).astype(np.float16)
    bia = np.ascontiguousarray(
        np.concatenate([bq.reshape(4, 128).T, bk.reshape(4, 128).T], axis=1)
    )
    idn = np.eye(128, dtype=np.float16)

    # strip band mask [128, NH*SB]: partition p = 32*s + p', band col b;
    # in-window iff 0 <= b - p' <= 2W  (global edges fixed on host)
    p = np.arange(128)
    pp = p % 32
    b_ = np.arange(SB)
    m0 = np.where(
        (b_[None, :] - pp[:, None] >= 0) & (b_[None, :] - pp[:, None] <= 2 * W),
        0.0,
        NEG,
    ).astype(np.float16)
    mk = np.tile(m0, (1, NH))  # [128, NH*SB]
    smi = np.ascontiguousarray(np.concatenate([mk, idn], axis=1))  # [128, 640]

    wq4 = wqT.reshape(4, 128, HID)  # [ic, p, o]
    wk4 = wkT.reshape(4, 128, HID)

    in_maps = []
    for c in range(NCORES):
        b, th = c // 2, c % 2
        t0 = th * THALF
        qTs = query[b].T[:, t0 : t0 + THALF].astype(np.float16)  # [HID, THALF]
        kTs = np.zeros((HID, KW), np.float16)
        j0 = t0 - W
        lo, hi = max(j0, 0), min(t0 + THALF + W, T)
        kTs[:, lo - j0 : hi - j0] = key[b].T[:, lo:hi].astype(np.float16)
        q4 = qTs.reshape(4, 128, THALF)  # [ic, p, t]
        k4 = kTs.reshape(4, 128, KW)
        in1 = np.empty((128, 4, 1024), np.float16)
        in1[:, :, 0:512] = wq4.transpose(1, 0, 2)
        in1[:, :, 512:1024] = q4[:, :, 0:512].transpose(1, 0, 2)
        in2 = np.empty((128, 4, 1024), np.float16)
        in2[:, :, 0:512] = wk4.transpose(1, 0, 2)
        in2[:, :, 512:1024] = k4[:, :, 0:512].transpose(1, 0, 2)
        in3 = np.empty((128, 4, 1056), np.float16)
        in3[:, :, 0:512] = q4[:, :, 512:1024].transpose(1, 0, 2)
        in3[:, :, 512:1056] = k4[:, :, 512:KW].transpose(1, 0, 2)
        in_maps.append(
            {
                "in1": np.ascontiguousarray(in1),
                "in2": np.ascontiguousarray(in2),
                "in3": np.ascontiguousarray(in3),
                "smi": smi,
                "bia": bia,
            }
        )
    return in_maps


def host_gather(results):
    """results: list of 8 dicts with 'outp' [128, NBLK, NH, SB] and
    'sums' [128, NBLK, NH] -> full output [B, NH, T, WIN]."""
    band = np.empty((B, NH, T, SB), np.float32)
    den = np.empty((B, NH, T), np.float32)
    for c in range(NCORES):
        b, th = c // 2, c % 2
        t0 = th * THALF
        # [p, r, h, n] -> [h, r, p, n] -> [h, r*128+p, n]
        band[b, :, t0 : t0 + THALF] = (
            results[c]["outp"].transpose(2, 1, 0, 3).reshape(NH, THALF, SB)
        )
        den[b, :, t0 : t0 + THALF] = (
            results[c]["sums"].transpose(2, 1, 0).reshape(NH, THALF)
        )
    # exact denominator correction for global edge rows: out-of-range keys
    # are zero-padded -> score 0 -> exp contributes exactly 1.0 each
    i = np.arange(T)
    n_inv = np.maximum(0, W - i) + np.maximum(0, i - (T - 1 - W))
    den -= n_inv[None, None, :].astype(np.float32)
    # gather the select window from the strip band
    g0 = np.clip(i - W, 0, T - WIN)
    c0 = g0 - i + (i % 32) + W  # start col within the 64-wide strip band
    idx = c0[:, None] + np.arange(WIN)[None, :]  # [T, WIN]
    out = np.take_along_axis(band, idx[None, None, :, :], axis=-1)
    out /= den[..., None]
    return np.ascontiguousarray(out)


def kernel(query, key, Wq, bq, Wk, bk):
    from concourse import bass_utils

    nc = _get_nc()
    in_maps = host_prep(query, key, Wq, bq, Wk, bk)
    res = bass_utils.run_bass_kernel_spmd(nc, in_maps, core_ids=list(range(NCORES)))
    return host_gather(res.results)


# revision 10
# speedup vs baseline: 1.0369x; 1.0369x over previous
"""Banded multi-head attention kernel for Trainium2 (8 NeuronCores).

Problem: q = query @ Wq.T + bq, k = key @ Wk.T + bk  (per head, dk=64),
scores = q.k / sqrt(dk) masked to |i-j| <= 16, softmax over keys, then
gather the 33-column select window per row -> out [B, NH, T, 33].

Strategy (v4):
  - Shard (batch b, half of T) across the 8 cores; each core computes all
    8 heads for its 1024 query rows.
  - All matmuls in float16 (1 cycle/col on the PE vs 4 for fp32
    LOW_HIGH multipass; fp32 PSUM accumulation).
  - Scores in a 64-wide band: each 128-row block is split into four
    32-row strips whose key windows are 64 wide.  8 heads x 4 strips =
    32 matmuls [K=64,M=32,N=64] pack into one 512-col PSUM bank via
    4-way column tiling (~4ns issue stagger, measured).  All score
    matmuls use PE array rows 0-63: mixing row groups across
    column-sharing matmuls faults the device (measured), so odd heads'
    projections (PSUM partitions 64-127) are moved to partitions 0-63
    with SBUF->SBUF DMAs.
  - The band mask initializes the PSUM bank via one matmul
    ident.T @ mask (start=True); the 32 strip matmuls accumulate on top
    (per-element has_written).  In-window mask = -C*TEMP so the exp band
    is scaled by e^-C and fits float16 (C=5; offset cancels in the
    softmax ratio).  ScalarE does exp((x-40)/8) PSUM->f16 SBUF in ONE
    512-wide op per block; DVE one 512-wide f16 row-sum reduce.
  - DMA: inputs packed as 3 flat [128, N] blobs (8KB/partition
    descriptors) + 2 small, split across both HW DGE queues (Sync +
    ScalarE); f16 band halves output bytes; 56 tiny N=64 warm-up
    matmuls bridge the DMA lead-in so HAM un-throttles the PE clock
    without delaying the first projection by more than one tiny MM.
  - Host: diagonal gather band -> [T, 33], exact edge-row denominator
    correction (invalid cols contribute exactly f16(exp(-C)) each),
    divide.
"""

import sys

sys.path.insert(0, "/opt/trn_rl_repo")

import numpy as np

B, T, HID = 4, 2048, 512
NH, DK, W = 8, 64, 16
WIN = 2 * W + 1  # 33
TEMP = 8.0
NCORES = 8
THALF = T // 2  # rows per core
NBLK = THALF // 128  # 8 row blocks per core
SB = 64  # strip band width: 32-row strip -> 32 + 2*16 keys
KW = THALF + 2 * W  # 1056 k^T columns needed per core
NEG = -60000.0  # out-of-window mask value (exp -> 0)
CEXP = 5.0  # exp offset: band stores e^(s/8 - C), cancels in softmax
NWARM = 56  # tiny HAM warm-up matmuls

_CACHE = {}


def _build_nc():
    import concourse.bass as bass  # noqa: F401
    import concourse.tile as tile
    from concourse import bacc, mybir

    f32 = mybir.dt.float32
    f16 = mybir.dt.float16
    AF = mybir.ActivationFunctionType
    AX = mybir.AxisListType
    ALU = mybir.AluOpType

    nc = bacc.Bacc("TRN2", target_bir_lowering=False, debug=False)

    # flat input blobs (8KB-per-partition contiguous DMA descriptors);
    # layout per partition: in1 = [wq(4x512) | q cols 0:512 (4x512)]
    # in2 = [wk | k cols 0:512], in3 = [q cols 512:1024 | k cols 512:1056]
    in1 = nc.dram_tensor("in1", [128, 4096], f16, kind="ExternalInput").ap()
    in2 = nc.dram_tensor("in2", [128, 4096], f16, kind="ExternalInput").ap()
    in3 = nc.dram_tensor("in3", [128, 4224], f16, kind="ExternalInput").ap()
    smi = nc.dram_tensor("smi", [128, 640], f16, kind="ExternalInput").ap()
    bia = nc.dram_tensor("bia", [128, 8], f32, kind="ExternalInput").ap()
    # exp band (f16, scaled by e^-C): [p, r, h, n]; row sums f32 [p, r, h]
    outp = nc.dram_tensor(
        "outp", [128, NBLK, NH, SB], f16, kind="ExternalOutput"
    ).ap()
    sums = nc.dram_tensor("sums", [128, NBLK, NH], f32, kind="ExternalOutput").ap()

    with tile.TileContext(nc) as tc:
        from contextlib import ExitStack

        with ExitStack() as ctx:
            const = ctx.enter_context(tc.tile_pool(name="const", bufs=1))
            psum_p = ctx.enter_context(
                tc.tile_pool(name="psum_p", bufs=3, space="PSUM")
            )
            psum_s = ctx.enter_context(
                tc.tile_pool(name="psum_s", bufs=3, space="PSUM")
            )
            psum_w = ctx.enter_context(
                tc.tile_pool(name="psum_w", bufs=1, space="PSUM")
            )

            in1_sb = const.tile([128, 4096], f16, tag="in1", name="in1")
            in2_sb = const.tile([128, 4096], f16, tag="in2", name="in2")
            in3_sb = const.tile([128, 4224], f16, tag="in3", name="in3")
            sm_sb = const.tile([128, 640], f16, tag="smi", name="smi")
            bia_sb = const.tile([128, 8], f32, tag="bia", name="bias")
            warm = const.tile([128, 192], f16, tag="wrm", name="wrm")
            # projections: [p = out-channel within oc chunk, oc, t]
            qp = const.tile([128, 4, THALF], f16, tag="qp", name="qp")
            kp = const.tile([128, 4, KW], f16, tag="kp", name="kp")
            # odd heads' dk rows moved to partitions 0-63
            qpo = const.tile([64, 4, THALF], f16, tag="qpo", name="qpo")
            kpo = const.tile([64, 4, KW], f16, tag="kpo", name="kpo")
            # persistent exp-band region [p, r, h, n] (f16) + row sums
            ob = const.tile([128, NBLK, NH, SB], f16, tag="ob", name="ob")
            rs_sb = const.tile([128, NBLK, NH], f32, tag="rs", name="rs")

            msk_ap = sm_sb[:, 0:512]
            idn_ap = sm_sb[:, 512:640]

            def wq_ap(ic, osl):
                return in1_sb[:, 1024 * ic + osl.start : 1024 * ic + osl.stop]

            def wk_ap(ic, osl):
                return in2_sb[:, 1024 * ic + osl.start : 1024 * ic + osl.stop]

            def qin_ap(ic, tb):  # tb 0 -> in1 cols 512:1024, 1 -> in3 0:512
                if tb == 0:
                    return in1_sb[:, 1024 * ic + 512 : 1024 * ic + 1024]
                return in3_sb[:, 1056 * ic : 1056 * ic + 512]

            def kin_ap(ic, c0, cn):  # c0=0 -> in2; else in3
                if c0 == 0:
                    return in2_sb[:, 1024 * ic + 512 : 1024 * ic + 512 + cn]
                off = 1056 * ic + 512 + (c0 - 512)
                return in3_sb[:, off : off + cn]

            # HAM warm-up: tiny dummy matmuls on a memset tile keep the PE
            # busy during the DMA lead-in so the clock gate opens before
            # the real matmuls start; each costs only ~50ns if data is
            # ready early.
            nc.gpsimd.memset(warm[:, :], 0.0)
            wps = psum_w.tile([128, 512], f32, tag="wps", name="wps")
            for _ in range(NWARM):
                nc.tensor.matmul(
                    wps[:, 0:64], warm[:, 0:128], warm[:, 128:192],
                    start=True, stop=True,
                )

            # input DMAs split across both hardware DGE queues,
            # ordered by first use
            nc.sync.dma_start(out=in1_sb[:, :], in_=in1[:, :])
            nc.scalar.dma_start(out=bia_sb[:, :], in_=bia[:, :])
            nc.scalar.dma_start(out=sm_sb[:, :], in_=smi[:, :])
            nc.scalar.dma_start(out=in2_sb[:, :], in_=in2[:, :])
            nc.sync.dma_start(out=in3_sb[:, :], in_=in3[:, :])

            ncopy = [0]

            def psum_to_sbuf(dst, ps_ap, bia_ap):
                # psum->sbuf(+bias) copies: ~1/3 ScalarE, 2/3 DVE
                if ncopy[0] % 3 == 0:
                    nc.scalar.activation(
                        dst, ps_ap, AF.Identity, bias=bia_ap, scale=1.0
                    )
                else:
                    nc.vector.tensor_scalar_add(dst, ps_ap, bia_ap)
                ncopy[0] += 1

            def emit_qproj(tb):
                tsl = slice(512 * tb, 512 * (tb + 1))
                for oc in range(4):
                    osl = slice(128 * oc, 128 * (oc + 1))
                    ps = psum_p.tile([128, 512], f32, tag="psp", name="psp")
                    for ic in range(4):
                        nc.tensor.matmul(
                            ps[:, :],
                            wq_ap(ic, osl),
                            qin_ap(ic, tb),
                            start=(ic == 0),
                            stop=(ic == 3),
                        )
                    psum_to_sbuf(qp[:, oc, tsl], ps[:, :], bia_sb[:, oc : oc + 1])
                # odd heads' 64 dk rows -> partitions 0-63 (all 4 oc at once)
                q = nc.sync if tb == 0 else nc.scalar
                q.dma_start(out=qpo[:, :, tsl], in_=qp[64:128, :, tsl])

            def emit_kproj(c0, cn):
                csl = slice(c0, c0 + cn)
                for oc in range(4):
                    osl = slice(128 * oc, 128 * (oc + 1))
                    ps = psum_p.tile([128, 512], f32, tag="psp", name="psp")
                    for ic in range(4):
                        nc.tensor.matmul(
                            ps[:, :cn],
                            wk_ap(ic, osl),
                            kin_ap(ic, c0, cn),
                            start=(ic == 0),
                            stop=(ic == 3),
                        )
                    psum_to_sbuf(
                        kp[:, oc, csl], ps[:, :cn], bia_sb[:, 4 + oc : 5 + oc]
                    )
                q = nc.scalar if c0 == 0 else nc.sync
                q.dma_start(out=kpo[:, :, csl], in_=kp[64:128, :, csl])

            def emit_scores(r):
                ps = psum_s.tile([128, NH * SB], f32, tag="pss", name="pss")
                # initialize the whole bank with the band mask via the PE
                # (ident.T @ mask, start=True); the 32 strip matmuls then
                # accumulate on top (per-element has_written adds onto the
                # mask).  skip_group_check: the sim's zero-region tracker
                # can't express partition-sliced accumulation, but
                # per-element HW semantics are exact (verified on HW).
                nc.tensor.matmul(
                    ps[:, :], idn_ap, msk_ap, start=True, stop=False,
                    skip_group_check=True,
                )
                nmm = 0
                for half in range(2):  # even heads first: qpo/kpo arrive later
                    for oc in range(4):
                        h = 2 * oc + half
                        for s in range(4):
                            c = 128 * r + 32 * s
                            if half == 0:
                                lhsT = qp[0:64, oc, c : c + 32]
                                rhs = kp[0:64, oc, c : c + SB]
                            else:
                                lhsT = qpo[:, oc, c : c + 32]
                                rhs = kpo[:, oc, c : c + SB]
                            nmm += 1
                            nc.tensor.matmul(
                                ps[32 * s : 32 * s + 32, SB * h : SB * (h + 1)],
                                lhsT,
                                rhs,
                                start=False,
                                stop=(nmm == 32),
                                tile_position=(0, 32 * s),
                                skip_group_check=True,
                            )
                nc.scalar.activation(
                    ob[:, r, :, :], ps[:, :], AF.Exp, scale=1.0 / TEMP
                )
                nc.vector.tensor_reduce(
                    rs_sb[:, r, :], ob[:, r, :, :], axis=AX.X, op=ALU.add
                )
                q = nc.sync if r % 2 == 1 else nc.scalar
                q.dma_start(out=outp[:, r, :, :], in_=ob[:, r, :, :])

            # interleave projections and score blocks so ScalarE/DVE
            # post-processing overlaps PE matmuls throughout
            emit_qproj(0)
            emit_kproj(0, 512)
            for r in range(0, 3):
                emit_scores(r)
            emit_qproj(1)
            emit_kproj(512, 512)
            for r in range(3, 7):
                emit_scores(r)
            emit_kproj(1024, KW - 1024)
            emit_scores(7)
            nc.sync.dma_start(out=sums[:, :, :], in_=rs_sb[:, :, :])

    nc.compile()
    return nc


def _get_nc():
    if "nc" not in _CACHE:
        _CACHE["nc"] = _build_nc()
    return _CACHE["nc"]


def host_prep(query, key, Wq, bq, Wk, bk):
    """Build the 8 per-core input maps."""
    query = np.asarray(query, dtype=np.float32)
    key = np.asarray(key, dtype=np.float32)
    Wq = np.asarray(Wq, dtype=np.float32)
    Wk = np.asarray(Wk, dtype=np.float32)
    bq = np.asarray(bq, dtype=np.float32)
    bk = np.asarray(bk, dtype=np.float32)

    wqT = np.ascontiguousarray(Wq.T).astype(np.float16)  # [HID(in), HID(out)]
    wkT = np.ascontiguousarray(Wk.T).astype(np.float16)
    bia = np.ascontiguousarray(
        np.concatenate([bq.reshape(4, 128).T, bk.reshape(4, 128).T], axis=1)
    )
    idn = np.eye(128, dtype=np.float16)

    # strip band mask [128, NH*SB]: partition p = 32*s + p', band col b;
    # in-window iff 0 <= b - p' <= 2W -> -C*TEMP (exp offset), else NEG
    p = np.arange(128)
    pp = p % 32
    b_ = np.arange(SB)
    m0 = np.where(
        (b_[None, :] - pp[:, None] >= 0) & (b_[None, :] - pp[:, None] <= 2 * W),
        -CEXP * TEMP,
        NEG,
    ).astype(np.float16)
    mk = np.tile(m0, (1, NH))  # [128, NH*SB]
    smi = np.ascontiguousarray(np.concatenate([mk, idn], axis=1))  # [128, 640]

    wq4 = wqT.reshape(4, 128, HID).transpose(1, 0, 2)  # [p, ic, o]
    wk4 = wkT.reshape(4, 128, HID).transpose(1, 0, 2)

    in_maps = []
    for c in range(NCORES):
        b, th = c // 2, c % 2
        t0 = th * THALF
        qTs = query[b].T[:, t0 : t0 + THALF].astype(np.float16)  # [HID, THALF]
        kTs = np.zeros((HID, KW), np.float16)
        j0 = t0 - W
        lo, hi = max(j0, 0), min(t0 + THALF + W, T)
        kTs[:, lo - j0 : hi - j0] = key[b].T[:, lo:hi].astype(np.float16)
        q4 = qTs.reshape(4, 128, THALF).transpose(1, 0, 2)  # [p, ic, t]
        k4 = kTs.reshape(4, 128, KW).transpose(1, 0, 2)
        in1 = np.empty((128, 4, 1024), np.float16)
        in1[:, :, 0:512] = wq4
        in1[:, :, 512:1024] = q4[:, :, 0:512]
        in2 = np.empty((128, 4, 1024), np.float16)
        in2[:, :, 0:512] = wk4
        in2[:, :, 512:1024] = k4[:, :, 0:512]
        in3 = np.empty((128, 4, 1056), np.float16)
        in3[:, :, 0:512] = q4[:, :, 512:1024]
        in3[:, :, 512:1056] = k4[:, :, 512:KW]
        in_maps.append(
            {
                "in1": np.ascontiguousarray(in1.reshape(128, 4096)),
                "in2": np.ascontiguousarray(in2.reshape(128, 4096)),
                "in3": np.ascontiguousarray(in3.reshape(128, 4224)),
                "smi": smi,
                "bia": bia,
            }
        )
    return in_maps


def host_gather(results):
    """results: list of 8 dicts with 'outp' f16 [128, NBLK, NH, SB] and
    'sums' f32 [128, NBLK, NH] -> full output [B, NH, T, WIN]."""
    band = np.empty((B, NH, T, SB), np.float32)
    den = np.empty((B, NH, T), np.float32)
    for c in range(NCORES):
        b, th = c // 2, c % 2
        t0 = th * THALF
        # [p, r, h, n] -> [h, r, p, n] -> [h, r*128+p, n]
        band[b, :, t0 : t0 + THALF] = (
            results[c]["outp"]
            .astype(np.float32)
            .transpose(2, 1, 0, 3)
            .reshape(NH, THALF, SB)
        )
        den[b, :, t0 : t0 + THALF] = (
            results[c]["sums"].transpose(2, 1, 0).reshape(NH, THALF)
        )
    # exact denominator correction for global edge rows: out-of-range keys
    # are zero-padded -> score 0 -> each contributes f16(exp(-C)) to the sum
    i = np.arange(T)
    n_inv = np.maximum(0, W - i) + np.maximum(0, i - (T - 1 - W))
    einv = np.float32(np.float16(np.exp(np.float32(-CEXP))))
    den -= (n_inv * einv)[None, None, :].astype(np.float32)
    # gather the select window from the strip band
    g0 = np.clip(i - W, 0, T - WIN)
    c0 = g0 - i + (i % 32) + W  # start col within the 64-wide strip band
    idx = c0[:, None] + np.arange(WIN)[None, :]  # [T, WIN]
    out = np.take_along_axis(band, idx[None, None, :, :], axis=-1)
    out /= den[..., None]
    return np.ascontiguousarray(out)


def kernel(query, key, Wq, bq, Wk, bk):
    from concourse import bass_utils

    nc = _get_nc()
    in_maps = host_prep(query, key, Wq, bq, Wk, bk)
    res = bass_utils.run_bass_kernel_spmd(nc, in_maps, core_ids=list(range(NCORES)))
    return host_gather(res.results)
